# revision 15
# baseline (speedup 1.0000x reference)
"""Bass/Trainium2 kernel for nn_ConvLayer (sparse-GP conv layer conditional).

Computes, for X:[64,1600], Z:[384,25], q_mu:[384,1], q_sqrt:[384,384]:
    patches = im2col(X)                       [N,P,L]   P=1296, L=25
    Kuf = rbf(Z, patches)                     [M, N*P]
    Kuu = rbf(Z,Z) + jitter*I
    A = Kuu^-1 Kuf
    mean = A^T q_mu
    diag = colwise  A^T (Lq Lq^T - Kuu) A
    var  = variance + diag
Returns (mean [N,P], var [N,P]).

Strategy: data-parallel over batch N across 8 NeuronCores (8 images/core).
Host precomputes the shared M x M quantities in float64 (W = Kuu^-1,
Tt = var^2 * W (Lq Lq^T - Kuu) W, wt = var * W q_mu) and replicates them.
Per patch column c the device computes
    sq_c  = Zaug2^T Paug2   (augmented matmul = ||z||^2+||p||^2-2 z.p)
    E_c   = exp(s * sq_c)                      (s = -0.5/ls^2)
    G'_j  = sum_{k<=j} TU_kj^T E_k             (half-symmetric form)
    pr_j  = E_j o G'_j
    m_j   = wt_j^T E_j ;  d_j = 2 * 1^T pr_j   (col-tiled 1-row matmuls)
Per-chunk partials (m_j, d_j) are DMA'd out and combined on the host:
mean = sum_j m_j ; var = variance + sum_j d_j.

TensorE: 3 sq + 6 G' + 6 col-tiled reduction matmuls per 512-col tile
(the 6 reductions run ~3-way concurrent via tile_position col groups).
ScalarE: one fused exp over the 3-bank sq PSUM tile (+ every other
tile's partial-bank evacuation). VectorE: the 3 elementwise products
(+ the other half of evacuations). DMAs batch 4 column tiles per
transfer to keep the sync queue light.
"""

import numpy as np
from contextlib import ExitStack
from numpy.lib.stride_tricks import sliding_window_view

import concourse.bass as bass
import concourse.mybir as mybir
import concourse.tile as tile
from concourse import bacc
from concourse.bass_utils import run_bass_kernel_spmd

# Problem constants (hardcoded per spec)
H = 40
WID = 40
KS = 5
HOUT = H - KS + 1            # 36
WOUT = WID - KS + 1          # 36
P = HOUT * WOUT              # 1296
L = KS * KS                  # 25
M = 384                      # inducing points
N = 64                       # batch
JITTER = 1e-6
NCORES = 8
NPC = N // NCORES            # images per core = 8
COLS = NPC * P               # patch columns per core = 10368
LA = L + 2                   # augmented contraction: patches, ||p||^2, ones

F32 = mybir.dt.float32
F32R = mybir.dt.float32r
MCH = M // 128               # 3 chunks of the M dim

# column tiles: keep every matmul free-dim >= 256 so float32r runs at
# full rate (19*512 + 2*320 = 10368)
TILES = [(i * 512, 512) for i in range(19)] + [(19 * 512, 320), (19 * 512 + 320, 320)]
# group tiles into super-blocks of 4 for batched DMA
SUPERS = []
i = 0
while i < len(TILES):
    grp = TILES[i:i + 4]
    SUPERS.append((grp[0][0], sum(f for _, f in grp), grp))
    i += 4


def _build_program(s_scale: float):
    """Build the SPMD single-core Bass program (same on all 8 cores)."""
    nc = bacc.Bacc("TRN2", target_bir_lowering=False, debug=False, num_devices=NCORES)

    d_paug = nc.dram_tensor("paug", [LA, COLS], F32R, kind="ExternalInput").ap()
    d_zaug = nc.dram_tensor("zaugt", [LA, M], F32R, kind="ExternalInput").ap()
    d_rt = nc.dram_tensor("rt", [128, MCH, M], F32R, kind="ExternalInput").ap()
    d_redw = nc.dram_tensor("redw", [128, 4], F32R, kind="ExternalInput").ap()
    # row 0: mean; rows 1-3: per-chunk half-diag partials (host sums)
    d_out = nc.dram_tensor("outp", [4, COLS], F32, kind="ExternalOutput").ap()

    with tile.TileContext(nc) as tc, ExitStack() as ctx:
        const = ctx.enter_context(tc.tile_pool(name="const", bufs=1))
        pa_pool = ctx.enter_context(tc.tile_pool(name="pa", bufs=2))
        e_pool = ctx.enter_context(tc.tile_pool(name="epool", bufs=2))
        pr_pool = ctx.enter_context(tc.tile_pool(name="prpool", bufs=2))
        ob_pool = ctx.enter_context(tc.tile_pool(name="obpool", bufs=2))
        ps_sq = ctx.enter_context(tc.tile_pool(name="psq", bufs=1, space="PSUM"))
        ps_g = ctx.enter_context(tc.tile_pool(name="psg", bufs=3, space="PSUM"))
        ps_red = ctx.enter_context(tc.tile_pool(name="psred", bufs=2, space="PSUM"))

        sb_zaug = const.tile([LA, M], F32R)
        nc.sync.dma_start(sb_zaug[:, :], d_zaug)
        sb_redw = const.tile([128, 4], F32R)
        nc.sync.dma_start(sb_redw[:, :], d_redw)
        sb_rt = const.tile([128, MCH, M], F32R)
        for k in range(MCH):
            nc.sync.dma_start(sb_rt[:, k, :], d_rt[:, k, :])

        tidx = 0
        for (s0, SW, grp) in SUPERS:
            sb_pa = pa_pool.tile([LA, SW], F32R)
            nc.sync.dma_start(sb_pa[:, :], d_paug[:, s0:s0 + SW])
            obm = ob_pool.tile([1, SW], F32)
            obd = ob_pool.tile([1, MCH, SW], F32)

            for (c0, F) in grp:
                lo = c0 - s0
                # ---- sq = Zaug2^T @ Paug2, 3 chunks into one 3-bank PSUM tile
                psq = ps_sq.tile([128, MCH, 512], F32)
                for k in range(MCH):
                    nc.tensor.matmul(
                        psq[:, k, 0:F],
                        lhsT=sb_zaug[:, k * 128:(k + 1) * 128],
                        rhs=sb_pa[:, lo:lo + F],
                        start=True, stop=True,
                    )
                # ---- E = exp(s * sq): one fused activation over 3 banks
                e3 = e_pool.tile([128, MCH, F], F32R)
                nc.scalar.activation(
                    e3[:, :, :], psq[:, :, 0:F],
                    mybir.ActivationFunctionType.Exp,
                    scale=float(s_scale),
                )

                # ---- half-symmetric G'_j + elementwise product into one
                # [128, MCH, F] SBUF tile (GpSimd reduces it over partitions)
                pr3 = pr_pool.tile([128, MCH, F], F32, tag="pr")
                for j in range(MCH):
                    gps = ps_g.tile([128, F], F32)
                    for k in range(j + 1):
                        nc.tensor.matmul(
                            gps[:, :],
                            lhsT=sb_rt[:, k, j * 128:(j + 1) * 128],
                            rhs=e3[:, k, :],
                            start=(k == 0), stop=(k == j),
                        )
                    nc.vector.tensor_mul(pr3[:, j, :], e3[:, j, :], gps[:, :])

                # mean: 3-matmul accumulation chain into one PSUM row
                pps = ps_red.tile([1, 512], F32)
                for j in range(MCH):
                    nc.tensor.matmul(
                        pps[0:1, 0:F],
                        lhsT=sb_redw[:, j:j + 1],
                        rhs=e3[:, j, :],
                        start=(j == 0), stop=(j == MCH - 1),
                    )
                nc.scalar.copy(obm[0:1, lo:lo + F], pps[0:1, 0:F])

                # half-diag partials: GpSimd partition reduction on SBUF
                nc.gpsimd.tensor_reduce(
                    obd[0:1, :, lo:lo + F],
                    pr3[:, :, :],
                    axis=mybir.AxisListType.C,
                    op=mybir.AluOpType.add,
                )
                tidx += 1

            # ---- batched DMA out: mean row + 3 half-diag partial rows
            nc.sync.dma_start(d_out[0:1, s0:s0 + SW], obm[0:1, :])
            nc.sync.dma_start(d_out[1:4, s0:s0 + SW], obd[0:1, :, :])

    nc.compile()
    return nc


def _host_prep(X, Z, q_mu, q_sqrt, variance, lengthscale):
    var = float(np.asarray(variance).reshape(-1)[0])
    ls = float(np.asarray(lengthscale).reshape(-1)[0])
    s = -0.5 / (ls * ls)

    Z64 = np.asarray(Z, np.float64)
    zz = (Z64 * Z64).sum(1)                                   # [M]
    sq = zz[:, None] + zz[None, :] - 2.0 * (Z64 @ Z64.T)
    np.maximum(sq, 0.0, out=sq)
    Kuu = var * np.exp(s * sq) + JITTER * np.eye(M)
    Wi = np.linalg.inv(Kuu)
    Lq = np.tril(np.asarray(q_sqrt, np.float64))
    SK = Lq @ Lq.T - Kuu
    Tt = (var * var) * (Wi @ SK @ Wi)                         # symmetric
    wt = var * (Wi @ np.asarray(q_mu, np.float64)[:, 0])      # [M]

    # Block upper-triangle (with halved diagonal blocks) of symmetric Tt:
    # G'_j = sum_{k<=j} TU_kj^T E_k needs lhsT rt[m,k,j] = TU[128k+m, j]
    TU = Tt.copy().reshape(MCH, 128, MCH, 128)
    for kb in range(MCH):
        for jb in range(MCH):
            if kb > jb:
                TU[kb, :, jb, :] = 0.0
            elif kb == jb:
                TU[kb, :, jb, :] *= 0.5
    TU = TU.reshape(M, M)
    rt = np.ascontiguousarray(
        TU.reshape(MCH, 128, M).transpose(1, 0, 2)
    ).astype(np.float32)

    # reduction weights: col j = wt chunk j (mean), col 3 = 2.0 (ds sum)
    redw = np.zeros((128, 4), np.float32)
    for k in range(MCH):
        redw[:, k] = wt.reshape(MCH, 128)[k]
    redw[:, 3] = 2.0

    zaugt = np.empty((LA, M), np.float32)
    zaugt[:L] = -2.0 * Z64.T
    zaugt[L] = 1.0
    zaugt[L + 1] = zz

    # patches + squared norms + ones, per core
    Ximg = np.asarray(X, np.float32).reshape(N, H, WID)
    pw = sliding_window_view(Ximg, (KS, KS), axis=(1, 2))     # [N,36,36,5,5]
    patches = pw.reshape(N, P, L)
    p2 = (patches.astype(np.float64) ** 2).sum(-1)            # [N,P]
    paug = np.empty((NCORES, LA, COLS), np.float32)
    for c in range(NCORES):
        blk = patches[c * NPC:(c + 1) * NPC].reshape(COLS, L)
        paug[c, :L] = blk.T
        paug[c, L] = p2[c * NPC:(c + 1) * NPC].reshape(-1)
        paug[c, L + 1] = 1.0

    return s, var, zaugt, rt, redw, paug


def kernel(X, Z, q_mu, q_sqrt, variance, lengthscale, _trace=False, _trace_kwargs=None):
    s, var, zaugt, rt, redw, paug = _host_prep(
        X, Z, q_mu, q_sqrt, variance, lengthscale)

    nc = _build_program(s)

    in_maps = [
        {"paug": np.ascontiguousarray(paug[c]),
         "zaugt": zaugt, "rt": rt, "redw": redw}
        for c in range(NCORES)
    ]
    res = run_bass_kernel_spmd(
        nc, in_maps, list(range(NCORES)),
        trace=_trace, **(_trace_kwargs or {}),
    )

    mean = np.empty((N, P), np.float32)
    varo = np.empty((N, P), np.float32)
    for c in range(NCORES):
        ob = res.results[c]["outp"]                 # [4, COLS]
        m = ob[0]
        v = var + 2.0 * (ob[1] + ob[2] + ob[3])
        mean[c * NPC:(c + 1) * NPC] = m.reshape(NPC, P)
        varo[c * NPC:(c + 1) * NPC] = v.reshape(NPC, P)
    if _trace:
        return (mean, varo), res
    return mean, varo


# revision 22
# speedup vs baseline: 20.4313x; 20.4313x over previous
"""Bass/Trainium2 kernel for nn_ConvLayer (sparse-GP conv layer conditional).

Computes, for X:[64,1600], Z:[384,25], q_mu:[384,1], q_sqrt:[384,384]:
    patches = im2col(X)                       [N,P,L]   P=1296, L=25
    Kuf = rbf(Z, patches)                     [M, N*P]
    Kuu = rbf(Z,Z) + jitter*I
    A = Kuu^-1 Kuf
    mean = A^T q_mu
    diag = colwise  A^T (Lq Lq^T - Kuu) A
    var  = variance + diag
Returns (mean [N,P], var [N,P]).

Strategy: data-parallel over batch N across 8 NeuronCores (8 images/core).
Host precomputes the shared M x M quantities in float64 (W = Kuu^-1,
Tt = var^2 * W (Lq Lq^T - Kuu) W, wt = var * W q_mu) and replicates them.
Per patch column c the device computes
    sq_c  = Zaug2^T Paug2   (augmented matmul = ||z||^2+||p||^2-2 z.p)
    E_c   = exp(s * sq_c)                      (s = -0.5/ls^2)
    G'_j  = sum_{k<=j} TU_kj^T E_k             (half-symmetric form)
    pr_j  = E_j o G'_j
    m_j   = wt_j^T E_j ;  d_j = 2 * 1^T pr_j   (col-tiled 1-row matmuls)
Per-chunk partials (m_j, d_j) are DMA'd out and combined on the host:
mean = sum_j m_j ; var = variance + sum_j d_j.

TensorE: 3 sq + 6 G' + 6 col-tiled reduction matmuls per 512-col tile
(the 6 reductions run ~3-way concurrent via tile_position col groups).
ScalarE: one fused exp over the 3-bank sq PSUM tile (+ every other
tile's partial-bank evacuation). VectorE: the 3 elementwise products
(+ the other half of evacuations). DMAs batch 4 column tiles per
transfer to keep the sync queue light.
"""

import numpy as np
from contextlib import ExitStack
from numpy.lib.stride_tricks import sliding_window_view

import concourse.bass as bass
import concourse.mybir as mybir
import concourse.tile as tile
from concourse import bacc
from concourse.bass_utils import run_bass_kernel_spmd

# Problem constants (hardcoded per spec)
H = 40
WID = 40
KS = 5
HOUT = H - KS + 1            # 36
WOUT = WID - KS + 1          # 36
P = HOUT * WOUT              # 1296
L = KS * KS                  # 25
M = 384                      # inducing points
N = 64                       # batch
JITTER = 1e-6
NCORES = 8
NPC = N // NCORES            # images per core = 8
COLS = NPC * P               # patch columns per core = 10368
LA = L + 2                   # augmented contraction: patches, ||p||^2, ones

F32 = mybir.dt.float32
F32R = mybir.dt.float32r
MCH = M // 128               # 3 chunks of the M dim

# column tiles: keep every matmul free-dim >= 256 so float32r runs at
# full rate (19*512 + 2*320 = 10368)
TILES = [(i * 512, 512) for i in range(19)] + [(19 * 512, 320), (19 * 512 + 320, 320)]
# group tiles into super-blocks of 4 for batched DMA
SUPERS = []
i = 0
while i < len(TILES):
    grp = TILES[i:i + 4]
    SUPERS.append((grp[0][0], sum(f for _, f in grp), grp))
    i += 4


def _build_program(s_scale: float):
    """Build the SPMD single-core Bass program (same on all 8 cores)."""
    nc = bacc.Bacc("TRN2", target_bir_lowering=False, debug=False, num_devices=NCORES)

    d_paug = nc.dram_tensor("paug", [LA, COLS], F32R, kind="ExternalInput").ap()
    d_zaug = nc.dram_tensor("zaugt", [LA, M], F32R, kind="ExternalInput").ap()
    d_rt = nc.dram_tensor("rt", [128, MCH, M], F32R, kind="ExternalInput").ap()
    d_redw = nc.dram_tensor("redw", [128, 8], F32R, kind="ExternalInput").ap()
    # row 0: mean; row 1: 2*half-diag (host adds variance)
    d_out = nc.dram_tensor("outp", [2, COLS], F32, kind="ExternalOutput").ap()

    with tile.TileContext(nc) as tc, ExitStack() as ctx:
        const = ctx.enter_context(tc.tile_pool(name="const", bufs=1))
        pa_pool = ctx.enter_context(tc.tile_pool(name="pa", bufs=2))
        e_pool = ctx.enter_context(tc.tile_pool(name="epool", bufs=2))
        pr_pool = ctx.enter_context(tc.tile_pool(name="prpool", bufs=2))
        ob_pool = ctx.enter_context(tc.tile_pool(name="obpool", bufs=2))
        ps_sq = ctx.enter_context(tc.tile_pool(name="psq", bufs=1, space="PSUM"))
        ps_g = ctx.enter_context(tc.tile_pool(name="psg", bufs=3, space="PSUM"))
        ps_red = ctx.enter_context(tc.tile_pool(name="psred", bufs=2, space="PSUM"))

        sb_zaug = const.tile([LA, M], F32R)
        nc.sync.dma_start(sb_zaug[:, :], d_zaug)
        sb_redw = const.tile([128, 8], F32R)
        nc.sync.dma_start(sb_redw[:, :], d_redw)
        sb_rt = const.tile([128, MCH, M], F32R)
        for k in range(MCH):
            nc.sync.dma_start(sb_rt[:, k, :], d_rt[:, k, :])

        tidx = 0
        for (s0, SW, grp) in SUPERS:
            sb_pa = pa_pool.tile([LA, SW], F32R)
            nc.sync.dma_start(sb_pa[:, :], d_paug[:, s0:s0 + SW])
            ob = ob_pool.tile([2, SW], F32)

            for (c0, F) in grp:
                lo = c0 - s0
                # ---- sq = Zaug2^T @ Paug2, 3 chunks into one 3-bank PSUM tile
                psq = ps_sq.tile([128, MCH, 512], F32)
                for k in range(MCH):
                    nc.tensor.matmul(
                        psq[:, k, 0:F],
                        lhsT=sb_zaug[:, k * 128:(k + 1) * 128],
                        rhs=sb_pa[:, lo:lo + F],
                        start=True, stop=True,
                    )
                # ---- E = exp(s * sq): one fused activation over 3 banks
                e3 = e_pool.tile([128, MCH, F], F32R)
                nc.scalar.activation(
                    e3[:, :, :], psq[:, :, 0:F],
                    mybir.ActivationFunctionType.Exp,
                    scale=float(s_scale),
                )

                # ---- half-symmetric G'_j + elementwise product + reductions.
                # One 6-step accumulation chain into a [2, F] PSUM row pair:
                # E_j steps add wt_j^T E_j to row 0 (mean), pr_j steps add
                # 2*1^T pr_j to row 1 (diag).
                pps = ps_red.tile([2, 512], F32)
                for j in range(MCH):
                    gps = ps_g.tile([128, F], F32)
                    for k in range(j + 1):
                        nc.tensor.matmul(
                            gps[:, :],
                            lhsT=sb_rt[:, k, j * 128:(j + 1) * 128],
                            rhs=e3[:, k, :],
                            start=(k == 0), stop=(k == j),
                        )
                    pr = pr_pool.tile([128, F], F32R, tag="pr")
                    nc.vector.tensor_mul(pr[:, :], e3[:, j, :], gps[:, :])
                    nc.tensor.matmul(
                        pps[0:2, 0:F],
                        lhsT=sb_redw[:, 2 * j:2 * j + 2],
                        rhs=e3[:, j, :],
                        start=(j == 0), stop=False,
                    )
                    nc.tensor.matmul(
                        pps[0:2, 0:F],
                        lhsT=sb_redw[:, 6:8],
                        rhs=pr[:, :],
                        start=False, stop=(j == MCH - 1),
                    )

                # evacuate the [2, F] partial rows (alternate engines)
                if tidx % 4 != 3:
                    nc.scalar.copy(ob[:, lo:lo + F], pps[0:2, 0:F])
                else:
                    nc.vector.tensor_copy(ob[:, lo:lo + F], pps[0:2, 0:F])
                tidx += 1

            # ---- batched DMA out: [mean, 2*half-diag] rows
            nc.sync.dma_start(d_out[0:2, s0:s0 + SW], ob[0:2, :])

    nc.compile()
    return nc


def _host_prep(X, Z, q_mu, q_sqrt, variance, lengthscale):
    var = float(np.asarray(variance).reshape(-1)[0])
    ls = float(np.asarray(lengthscale).reshape(-1)[0])
    s = -0.5 / (ls * ls)

    Z64 = np.asarray(Z, np.float64)
    zz = (Z64 * Z64).sum(1)                                   # [M]
    sq = zz[:, None] + zz[None, :] - 2.0 * (Z64 @ Z64.T)
    np.maximum(sq, 0.0, out=sq)
    Kuu = var * np.exp(s * sq) + JITTER * np.eye(M)
    Wi = np.linalg.inv(Kuu)
    Lq = np.tril(np.asarray(q_sqrt, np.float64))
    SK = Lq @ Lq.T - Kuu
    Tt = (var * var) * (Wi @ SK @ Wi)                         # symmetric
    wt = var * (Wi @ np.asarray(q_mu, np.float64)[:, 0])      # [M]

    # Block upper-triangle (with halved diagonal blocks) of symmetric Tt:
    # G'_j = sum_{k<=j} TU_kj^T E_k needs lhsT rt[m,k,j] = TU[128k+m, j]
    TU = Tt.copy().reshape(MCH, 128, MCH, 128)
    for kb in range(MCH):
        for jb in range(MCH):
            if kb > jb:
                TU[kb, :, jb, :] = 0.0
            elif kb == jb:
                TU[kb, :, jb, :] *= 0.5
    TU = TU.reshape(M, M)
    rt = np.ascontiguousarray(
        TU.reshape(MCH, 128, M).transpose(1, 0, 2)
    ).astype(np.float32)

    # reduction weights: cols 2j = [wt_j, 0] pairs; cols 6:8 = [0, 2.0]
    redw = np.zeros((128, 8), np.float32)
    for k in range(MCH):
        redw[:, 2 * k] = wt.reshape(MCH, 128)[k]
    redw[:, 7] = 2.0

    zaugt = np.empty((LA, M), np.float32)
    zaugt[:L] = -2.0 * Z64.T
    zaugt[L] = 1.0
    zaugt[L + 1] = zz

    # patches + squared norms + ones, per core
    Ximg = np.asarray(X, np.float32).reshape(N, H, WID)
    pw = sliding_window_view(Ximg, (KS, KS), axis=(1, 2))     # [N,36,36,5,5]
    patches = pw.reshape(N, P, L)
    p2 = (patches.astype(np.float64) ** 2).sum(-1)            # [N,P]
    paug = np.empty((NCORES, LA, COLS), np.float32)
    for c in range(NCORES):
        blk = patches[c * NPC:(c + 1) * NPC].reshape(COLS, L)
        paug[c, :L] = blk.T
        paug[c, L] = p2[c * NPC:(c + 1) * NPC].reshape(-1)
        paug[c, L + 1] = 1.0

    return s, var, zaugt, rt, redw, paug


def kernel(X, Z, q_mu, q_sqrt, variance, lengthscale, _trace=False, _trace_kwargs=None):
    s, var, zaugt, rt, redw, paug = _host_prep(
        X, Z, q_mu, q_sqrt, variance, lengthscale)

    nc = _build_program(s)

    in_maps = [
        {"paug": np.ascontiguousarray(paug[c]),
         "zaugt": zaugt, "rt": rt, "redw": redw}
        for c in range(NCORES)
    ]
    res = run_bass_kernel_spmd(
        nc, in_maps, list(range(NCORES)),
        trace=_trace, **(_trace_kwargs or {}),
    )

    mean = np.empty((N, P), np.float32)
    varo = np.empty((N, P), np.float32)
    for c in range(NCORES):
        ob = res.results[c]["outp"]                 # [2, COLS]
        mean[c * NPC:(c + 1) * NPC] = ob[0].reshape(NPC, P)
        varo[c * NPC:(c + 1) * NPC] = (var + ob[1]).reshape(NPC, P)
    if _trace:
        return (mean, varo), res
    return mean, varo
